# revision 10
# baseline (speedup 1.0000x reference)
# kernel.py — nn_CustomLinearEval: group-dequantized linear layer on 8 trn2 cores.
#
# out[b,s,n] = sum_k x[b,s,k] * w_dq[k,n] + bias[n]
#   w_dq = round(weight.T / s) * s,  s = step_scales[g,n] + 1e-8, g = k // 128
#
# Sharding: tensor-parallel over N (8 x 512 out-channels). The PE matmul work
# (1.05M cycles/core @ 1 elem/cycle, 2.4 GHz => ~437us) is the roofline;
# everything else is engineered to hide behind it:
#   - host sends wm = w.T*(1/s) + MAGIC (fp32 [K, 512] shard): mult + magic-add
#     are IEEE fp32 on host == identical to what the DVE would compute.
#   - device dequant, [k, n] orientation so NO on-device transposes:
#     q16 = (wm - MAGIC) via DVE tensor_scalar (round-half-even, small ints,
#     exact in fp16); w_dq16 = q16 * s_bcast via all-fp16 DVE tensor_tensor.
#     Processed in 8 slabs of 4 k-tiles, each fed by ONE batched 3D-AP DMA
#     (the SP sequencer spends ~0.6us configuring every DMA trigger, so few
#     big DMAs beat many small ones in the prologue). Scale rows reach SBUF
#     via a partition-stride-0 broadcast DMA (reads 4KB, writes 128 rows).
#   - matmul: out^T[n=128,m] += wdqT16[k,n].T @ x^T[k,m], fp16 operands, fp32
#     PSUM accumulate over 32 k-tiles. m-blocks of 1024 split into [128,512]
#     sub-accumulators (1 PSUM bank each, 2 generations x 4 = all 8 banks);
#     m-block 0 interleaves all 8 chains k-major (consumption paced to the
#     dequant pipeline), later m-blocks run 2-way interleaved halves; the
#     narrow sub-accumulators let evictions pipeline behind the chain stops,
#     shrinking the drain tail.
#   - x^T fp16 streamed per m-block as 8 double-buffered part-tiles, one
#     batched DMA per part; every core streams the full x.
#   - PE warmed up with a few dummy matmuls so the p-state reaches 2.4 GHz
#     right as the real stream starts; bias-add fused into the PSUM->SBUF
#     eviction on the scalar engine.
# Host gathers the 8 out^T shards ([512, 8192] each) and transposes once.

import numpy as np

GS = 128
EPS = 1e-8
B, S, K, N = 4, 2048, 4096, 4096
M = B * S
NCORES = 8
NS = N // NCORES          # 512 out-channels per core
G = K // GS               # 32 quant groups == k-tiles
KT = K // 128             # 32
NT = NS // 128            # 4 n-tiles per core
MB = 1024                 # m-block size
NMB = M // MB             # 8
SLAB = 4                  # k-tiles per dequant slab / x part-tile
NSLAB = KT // SLAB        # 8
NWARM = 12                # PE p-state warmup matmuls
MAGIC = float(np.float32(12582912.0))  # 1.5 * 2**23: fp32 round-half-even trick

_NC_CACHE = {}


def _build_nc():
    import concourse.bass as bass
    import concourse.mybir as mybir
    import concourse.tile as tile

    f32 = mybir.dt.float32
    f16 = mybir.dt.float16
    AF = mybir.ActivationFunctionType
    OP = mybir.AluOpType

    nc = bass.Bass()
    # host-pretransposed fp16 x: [K, M], full (every core reads all of it)
    xt16 = nc.dram_tensor("xt16", [K, M], f16, kind="ExternalInput")
    # wm = w.T * recip + MAGIC, fp32 shard [K, NS]
    wm = nc.dram_tensor("wm", [K, NS], f32, kind="ExternalInput")
    # s_eff shard as a single row [1, G*NS]; broadcast to 128 partitions by DMA
    srow16 = nc.dram_tensor("srow16", [1, G * NS], f16, kind="ExternalInput")
    # bias shard in [128, NT] layout (col nt, partition p -> bias[nt*128+p])
    brep = nc.dram_tensor("brep", [128, NT], f32, kind="ExternalInput")
    out_t = nc.dram_tensor("out_t", [NS, M], f32, kind="ExternalOutput")

    def x_part_ap(s, mb):
        # [128p, SLAB kt, MB j] view of xt16 rows s*SLAB*128.., cols mb*MB..
        base = xt16[0:128, 0:MB]
        off = (s * SLAB * 128) * M + mb * MB
        return bass.AP(base.tensor, off, [[M, 128], [128 * M, SLAB], [1, MB]])

    def wm_slab_ap(s):
        base = wm[0:128, 0:NS]
        off = (s * SLAB * 128) * NS
        return bass.AP(base.tensor, off, [[NS, 128], [128 * NS, SLAB], [1, NS]])

    def s_bcast_ap(s):
        # partition-stride-0 broadcast of srow16 slab s to 128 partitions
        base = srow16[0:1, :]
        return bass.AP(base.tensor, s * SLAB * NS, [[0, 128], [1, SLAB * NS]])

    with tile.TileContext(nc) as tc:
        with (
            tc.tile_pool(name="const", bufs=1) as constp,
            tc.tile_pool(name="wmp", bufs=2) as wmp,
            tc.tile_pool(name="sbb", bufs=2) as sbp,
            tc.tile_pool(name="tq", bufs=2) as tqp,
            tc.tile_pool(name="wdqT", bufs=1) as wdqp,
            tc.tile_pool(name="xp", bufs=2) as xp,
            tc.tile_pool(name="outsb", bufs=4) as outp,
            tc.tile_pool(name="acc", bufs=2, space="PSUM") as accp,
        ):
            b_sb = constp.tile([128, NT], f32)
            nc.sync.dma_start(b_sb[:], brep[:, :])
            dummy = constp.tile([128, 512], f16)
            nc.vector.memset(dummy[:], 0.0)

            # PSUM: 2 generations x 4 tags x [128,512] (1 bank) = all 8 banks
            def make_accs(n, mb):
                return [
                    accp.tile([128, 512], f32, tag=f"q{i % 4}", name=f"acc{mb}_{i}")
                    for i in range(n)
                ]

            accs0 = make_accs(8, 0)

            # p-state warmup: garbage matmuls keep the PE busy through the
            # prologue so the real stream starts at full clock.
            for i in range(NWARM):
                nc.tensor.matmul(
                    accs0[0][:], dummy[:, 0:128], dummy[:, 0:512],
                    start=True, stop=True, skip_group_check=True,
                )

            # dequantized weight shard, fp16 [k, n]: 8 slab tiles
            wdqT = [
                wdqp.tile([128, SLAB * NS], f16, name=f"wdqT{s}")
                for s in range(NSLAB)
            ]
            # x part-tiles for m-block 0
            xparts = [
                xp.tile([128, SLAB * MB], f16, tag=f"x{s}", name=f"x0_{s}")
                for s in range(NSLAB)
            ]

            # ---- prologue: per slab, batched DMAs + 2-op dequant
            for s in range(NSLAB):
                wm_t = wmp.tile([128, SLAB * NS], f32, tag="wm")
                nc.sync.dma_start(wm_t[:], wm_slab_ap(s))
                s_b = sbp.tile([128, SLAB * NS], f16, tag="sb")
                nc.sync.dma_start(s_b[:], s_bcast_ap(s))
                nc.sync.dma_start(xparts[s][:], x_part_ap(s, 0))
                # tq = wm - MAGIC = round(w/s): small integers, exact in fp16
                tq = tqp.tile([128, SLAB * NS], f16, tag="tq")
                nc.vector.tensor_scalar(tq[:], wm_t[:], MAGIC, None, op0=OP.subtract)
                nc.vector.tensor_tensor(wdqT[s][:], tq[:], s_b[:], op=OP.mult)

            def mm_one(acc_t, kt, nt, mh, first, last):
                s, ki = divmod(kt, SLAB)
                lhsT = wdqT[s][:, ki * NS + nt * 128 : ki * NS + (nt + 1) * 128]
                nc.tensor.matmul(
                    acc_t[:],
                    lhsT,
                    xparts[s][:, ki * MB + mh * 512 : ki * MB + (mh + 1) * 512],
                    start=first,
                    stop=last,
                )

            def evict_sub(acc_t, nt, mb, mh):
                o = outp.tile([128, 512], f32, tag="out", name=f"o{mb}_{nt}_{mh}")
                nc.scalar.activation(
                    o[:], acc_t[:], AF.Identity,
                    bias=b_sb[:, nt : nt + 1], scale=1.0,
                )
                nc.sync.dma_start(
                    out_t[
                        nt * 128 : (nt + 1) * 128,
                        mb * MB + mh * 512 : mb * MB + (mh + 1) * 512,
                    ],
                    o[:],
                )

            def refresh_x(mb):
                # issue next m-block's x DMAs (other buffer generation)
                parts = [
                    xp.tile([128, SLAB * MB], f16, tag=f"x{s}", name=f"x{mb}_{s}")
                    for s in range(NSLAB)
                ]
                for s in range(NSLAB):
                    nc.sync.dma_start(parts[s][:], x_part_ap(s, mb))
                return parts

            # ---- m-block 0: all 8 chains (4 n-tiles x 2 m-halves) interleaved
            # k-major so the PE consumes each wdqT slab as the dequant pipeline
            # emits it.
            chains0 = [(nt, mh) for nt in range(NT) for mh in range(2)]
            next_parts = refresh_x(1)
            for kt in range(KT):
                for c, (nt, mh) in enumerate(chains0):
                    mm_one(accs0[c], kt, nt, mh, kt == 0, kt == KT - 1)
            for c, (nt, mh) in enumerate(chains0):
                evict_sub(accs0[c], nt, 0, mh)
            xparts = next_parts

            # ---- m-blocks 1..NMB-1: 2-way interleaved halves
            for mb in range(1, NMB):
                if mb < NMB - 1:
                    next_parts = refresh_x(mb + 1)
                for half in range(2):
                    nts = (0, 1) if half == 0 else (2, 3)
                    chains = [(nt, mh) for nt in nts for mh in range(2)]
                    accs = make_accs(4, f"{mb}_{half}")
                    for kt in range(KT):
                        for c, (nt, mh) in enumerate(chains):
                            mm_one(accs[c], kt, nt, mh, kt == 0, kt == KT - 1)
                    for c, (nt, mh) in enumerate(chains):
                        evict_sub(accs[c], nt, mb, mh)
                if mb < NMB - 1:
                    xparts = next_parts

    _split_waits(nc)
    return nc


def _split_waits(nc, max_waits=1):
    """The walrus build in this container rejects >1 sync-wait per instruction
    ("Too many sync wait commands"). Hoist extra waits onto preceding
    same-engine NOPs, which is semantically identical (in-order engines)."""
    import concourse.mybir as mybir

    for func in nc.m.functions:
        for bb in func.blocks:
            insts = list(bb.instructions)
            new_insts = []
            changed = False
            for inst in insts:
                si = inst.sync_info
                waits = list(si.on_wait) if si is not None and si.on_wait else []
                if len(waits) > max_waits:
                    keep = waits[-max_waits:]
                    for j, wcond in enumerate(waits[:-max_waits]):
                        new_insts.append(
                            mybir.InstNoOp(
                                name=f"{inst.name}-ws{j}",
                                engine=inst.engine,
                                sync_info=mybir.SyncInfo(on_wait=[wcond], on_update=[]),
                            )
                        )
                    si.on_wait = keep
                    inst.sync_info = si
                    changed = True
                new_insts.append(inst)
            if changed:
                bb.instructions = new_insts


def _prep_inputs(x, weight, bias, step_scales):
    x = np.asarray(x, dtype=np.float32).reshape(M, K)
    weight = np.asarray(weight, dtype=np.float32)
    bias = np.asarray(bias, dtype=np.float32)
    step_scales = np.asarray(step_scales, dtype=np.float32)

    xt16 = np.ascontiguousarray(x.T.astype(np.float16))            # [K, M]

    s_eff = (step_scales + np.float32(EPS)).astype(np.float32)     # [G, N]
    recip = (np.float32(1.0) / s_eff).astype(np.float32)           # [G, N]
    # wm[k, n] = w.T[k, n] * recip[k//GS, n] + MAGIC, all IEEE fp32 — matches
    # the arithmetic the DVE would do, so rounding is bit-identical.
    w_t = np.ascontiguousarray(weight.T).reshape(G, GS, N)         # [G, GS, N]
    wm_full = (w_t * recip[:, None, :] + np.float32(MAGIC)).astype(np.float32)
    wm_full = wm_full.reshape(K, N)

    s16 = s_eff.astype(np.float16)                                 # [G, N]
    in_maps = []
    for c in range(NCORES):
        sl = slice(c * NS, (c + 1) * NS)
        in_maps.append(
            {
                "xt16": xt16,
                "wm": np.ascontiguousarray(wm_full[:, sl]),
                "srow16": np.ascontiguousarray(s16[:, sl].reshape(1, G * NS)),
                "brep": np.ascontiguousarray(bias[sl].reshape(NT, 128).T),
            }
        )
    return in_maps


def run_on_hw(x, weight, bias, step_scales, trace=False, **kw):
    from concourse.bass_utils import run_bass_kernel_spmd

    if "nc" not in _NC_CACHE:
        _NC_CACHE["nc"] = _build_nc()
    nc = _NC_CACHE["nc"]
    in_maps = _prep_inputs(x, weight, bias, step_scales)
    res = run_bass_kernel_spmd(
        nc, in_maps, core_ids=list(range(NCORES)), trace=trace, **kw
    )
    out_t = np.concatenate([res.results[c]["out_t"] for c in range(NCORES)], axis=0)
    out = np.ascontiguousarray(out_t.T).reshape(B, S, N)
    return out, res


def kernel(x, weight, bias, step_scales):
    out, _ = run_on_hw(x, weight, bias, step_scales, trace=False)
    return out


# revision 11
# speedup vs baseline: 1.1747x; 1.1747x over previous
# kernel.py — nn_CustomLinearEval: group-dequantized linear layer on 8 trn2 cores.
#
# out[b,s,n] = sum_k x[b,s,k] * w_dq[k,n] + bias[n]
#   w_dq = round(weight.T / s) * s,  s = step_scales[g,n] + 1e-8, g = k // 128
#
# Sharding: tensor-parallel over N (8 x 512 out-channels). The PE matmul work
# (1.05M cycles/core @ 1 elem/cycle, 2.4 GHz) is the roofline; everything else
# is engineered to hide behind it:
#   - host sends wm = w.T*(1/s) + MAGIC (fp32 [K, 512] shard): mult + magic-add
#     are IEEE fp32 on host == identical to what the DVE would compute.
#   - device dequant, [k, n] orientation so NO on-device transposes:
#     q16 = (wm - MAGIC) via DVE tensor_scalar (round-half-even, small ints,
#     exact in fp16); w_dq16 = q16 * s_bcast via all-fp16 DVE tensor_tensor.
#     Processed in 8 slabs of 4 k-tiles, each fed by ONE batched 3D-AP DMA
#     (SP sequencer spends ~0.6us configuring every DMA trigger, so few big
#     DMAs beat many small ones in the prologue).
#   - matmul: out^T[n=128,m] += wdqT16[k,n].T @ x^T[k,m], fp16 operands, fp32
#     PSUM accumulate over 32 k-tiles. m-blocks of 1024; m-block 0 interleaves
#     all 4 n-tile chains k-major (consumption paced to the dequant pipeline),
#     later m-blocks run 2-way interleaved halves with PSUM double-buffering.
#   - x^T fp16 streamed per m-block as 8 part-tiles (4 k-tiles each), double
#     buffered, one batched DMA per part; every core streams the full x.
#   - PE warmed up with dummy matmuls during the prologue so the p-state is
#     at 2.4 GHz when the real stream starts; bias-add fused into PSUM->SBUF
#     eviction on the scalar engine; final evictions chunked to shorten the
#     drain tail.
# Host gathers the 8 out^T shards ([512, 8192] each) and transposes once.

import numpy as np

GS = 128
EPS = 1e-8
B, S, K, N = 4, 2048, 4096, 4096
M = B * S
NCORES = 8
NS = N // NCORES          # 512 out-channels per core
G = K // GS               # 32 quant groups == k-tiles
KT = K // 128             # 32
NT = NS // 128            # 4 n-tiles per core
MB = 1024                 # m-block size
NMB = M // MB             # 8
SLAB = 4                  # k-tiles per dequant slab / x part-tile
NSLAB = KT // SLAB        # 8
NWARM = 12                # PE p-state warmup matmuls
MAGIC = float(np.float32(12582912.0))  # 1.5 * 2**23: fp32 round-half-even trick

_NC_CACHE = {}


def _build_nc():
    import concourse.bass as bass
    import concourse.mybir as mybir
    import concourse.tile as tile

    f32 = mybir.dt.float32
    f16 = mybir.dt.float16
    AF = mybir.ActivationFunctionType
    OP = mybir.AluOpType

    nc = bass.Bass()
    # host-pretransposed fp16 x: [K, M], full (every core reads all of it)
    xt16 = nc.dram_tensor("xt16", [K, M], f16, kind="ExternalInput")
    # wm = w.T * recip + MAGIC, fp32 shard [K, NS]
    wm = nc.dram_tensor("wm", [K, NS], f32, kind="ExternalInput")
    # s_eff shard as a single row [1, G*NS]; broadcast to 128 partitions by DMA
    srow16 = nc.dram_tensor("srow16", [1, G * NS], f16, kind="ExternalInput")
    # bias shard in [128, NT] layout (col nt, partition p -> bias[nt*128+p])
    brep = nc.dram_tensor("brep", [128, NT], f32, kind="ExternalInput")
    out_t = nc.dram_tensor("out_t", [NS, M], f32, kind="ExternalOutput")

    def x_part_ap(s, mb):
        # [128p, SLAB kt, MB j] view of xt16 rows s*SLAB*128.., cols mb*MB..
        base = xt16[0:128, 0:MB]
        off = (s * SLAB * 128) * M + mb * MB
        return bass.AP(base.tensor, off, [[M, 128], [128 * M, SLAB], [1, MB]])

    def wm_slab_ap(s):
        base = wm[0:128, 0:NS]
        off = (s * SLAB * 128) * NS
        return bass.AP(base.tensor, off, [[NS, 128], [128 * NS, SLAB], [1, NS]])

    def s_bcast_ap(s):
        # partition-stride-0 broadcast of srow16 slab s to 128 partitions
        base = srow16[0:1, :]
        return bass.AP(base.tensor, s * SLAB * NS, [[0, 128], [1, SLAB * NS]])

    with tile.TileContext(nc) as tc:
        with (
            tc.tile_pool(name="const", bufs=1) as constp,
            tc.tile_pool(name="wmp", bufs=2) as wmp,
            tc.tile_pool(name="sbb", bufs=2) as sbp,
            tc.tile_pool(name="tq", bufs=2) as tqp,
            tc.tile_pool(name="wdqT", bufs=1) as wdqp,
            tc.tile_pool(name="xp", bufs=2) as xp,
            tc.tile_pool(name="outsb", bufs=2) as outp,
            tc.tile_pool(name="outc", bufs=2) as outcp,
            tc.tile_pool(name="acc", bufs=2, space="PSUM") as accp,
        ):
            b_sb = constp.tile([128, NT], f32)
            nc.sync.dma_start(b_sb[:], brep[:, :])
            dummy = constp.tile([128, 512], f16)
            nc.vector.memset(dummy[:], 0.0)

            # PSUM accumulators: 2 generations x 2 tags x 2 banks = all 8 banks
            accs0 = [
                accp.tile([128, MB], f32, tag=f"a{i % 2}", name=f"acc0_{i}")
                for i in range(4)
            ]

            # p-state warmup: garbage matmuls keep the PE busy through the
            # prologue so the real stream starts at full clock.
            for i in range(NWARM):
                nc.tensor.matmul(
                    accs0[0][:, 0:512], dummy[:, 0:128], dummy[:, 0:512],
                    start=True, stop=True, skip_group_check=True,
                )

            # dequantized weight shard, fp16 [k, n]: 8 slab tiles
            wdqT = [
                wdqp.tile([128, SLAB * NS], f16, name=f"wdqT{s}")
                for s in range(NSLAB)
            ]
            # x part-tiles for m-block 0
            xparts = [
                xp.tile([128, SLAB * MB], f16, tag=f"x{s}", name=f"x0_{s}")
                for s in range(NSLAB)
            ]

            # ---- prologue: per slab, batched DMAs + 2-op dequant
            for s in range(NSLAB):
                wm_t = wmp.tile([128, SLAB * NS], f32, tag="wm")
                nc.sync.dma_start(wm_t[:], wm_slab_ap(s))
                s_b = sbp.tile([128, SLAB * NS], f16, tag="sb")
                nc.sync.dma_start(s_b[:], s_bcast_ap(s))
                nc.sync.dma_start(xparts[s][:], x_part_ap(s, 0))
                # tq = wm - MAGIC = round(w/s): small integers, exact in fp16
                tq = tqp.tile([128, SLAB * NS], f16, tag="tq")
                nc.vector.tensor_scalar(tq[:], wm_t[:], MAGIC, None, op0=OP.subtract)
                nc.vector.tensor_tensor(wdqT[s][:], tq[:], s_b[:], op=OP.mult)

            def mm_pair(acc_t, kt, nt, first, last):
                s, ki = divmod(kt, SLAB)
                lhsT = wdqT[s][:, ki * NS + nt * 128 : ki * NS + (nt + 1) * 128]
                rhs = xparts[s]
                nc.tensor.matmul(
                    acc_t[:, 0:512],
                    lhsT,
                    rhs[:, ki * MB : ki * MB + 512],
                    start=first,
                    stop=last,
                )
                nc.tensor.matmul(
                    acc_t[:, 512:MB],
                    lhsT,
                    rhs[:, ki * MB + 512 : (ki + 1) * MB],
                    start=first,
                    stop=last,
                )

            def evict(acc_t, nt, mb, chunks=1):
                cw = MB // chunks
                for c in range(chunks):
                    if chunks == 1:
                        o = outp.tile([128, MB], f32, tag="out", name=f"o{mb}_{nt}")
                        osl = o[:]
                    else:
                        o = outcp.tile([128, cw], f32, tag="oc", name=f"oc{nt}_{c}")
                        osl = o[:]
                    nc.scalar.activation(
                        osl, acc_t[:, c * cw : (c + 1) * cw], AF.Identity,
                        bias=b_sb[:, nt : nt + 1], scale=1.0,
                    )
                    nc.sync.dma_start(
                        out_t[
                            nt * 128 : (nt + 1) * 128,
                            mb * MB + c * cw : mb * MB + (c + 1) * cw,
                        ],
                        osl,
                    )

            def refresh_x(mb):
                # issue next m-block's x DMAs (other buffer generation)
                parts = [
                    xp.tile([128, SLAB * MB], f16, tag=f"x{s}", name=f"x{mb}_{s}")
                    for s in range(NSLAB)
                ]
                for s in range(NSLAB):
                    nc.sync.dma_start(parts[s][:], x_part_ap(s, mb))
                return parts

            # ---- m-block 0: all 4 n-tile chains interleaved k-major so the
            # PE consumes each wdqT slab right as the dequant pipeline emits it.
            next_parts = refresh_x(1)
            for kt in range(KT):
                for nt in range(NT):
                    mm_pair(accs0[nt], kt, nt, kt == 0, kt == KT - 1)
            for nt in range(NT):
                evict(accs0[nt], nt, 0)
            xparts = next_parts

            # ---- m-blocks 1..NMB-1: 2-way interleaved halves
            for mb in range(1, NMB):
                if mb < NMB - 1:
                    next_parts = refresh_x(mb + 1)
                for half in range(2):
                    nts = (0, 1) if half == 0 else (2, 3)
                    last_half = mb == NMB - 1 and half == 1
                    acc_a = accp.tile([128, MB], f32, tag="a0", name=f"am{mb}_{half}a")
                    acc_b = accp.tile([128, MB], f32, tag="a1", name=f"am{mb}_{half}b")
                    for kt in range(KT):
                        mm_pair(acc_a, kt, nts[0], kt == 0, kt == KT - 1)
                        mm_pair(acc_b, kt, nts[1], kt == 0, kt == KT - 1)
                    evict(acc_a, nts[0], mb, chunks=2 if last_half else 1)
                    evict(acc_b, nts[1], mb, chunks=2 if last_half else 1)
                if mb < NMB - 1:
                    xparts = next_parts

    _split_waits(nc)
    return nc


def _split_waits(nc, max_waits=1):
    """The walrus build in this container rejects >1 sync-wait per instruction
    ("Too many sync wait commands"). Hoist extra waits onto preceding
    same-engine NOPs, which is semantically identical (in-order engines)."""
    import concourse.mybir as mybir

    for func in nc.m.functions:
        for bb in func.blocks:
            insts = list(bb.instructions)
            new_insts = []
            changed = False
            for inst in insts:
                si = inst.sync_info
                waits = list(si.on_wait) if si is not None and si.on_wait else []
                if len(waits) > max_waits:
                    keep = waits[-max_waits:]
                    for j, wcond in enumerate(waits[:-max_waits]):
                        new_insts.append(
                            mybir.InstNoOp(
                                name=f"{inst.name}-ws{j}",
                                engine=inst.engine,
                                sync_info=mybir.SyncInfo(on_wait=[wcond], on_update=[]),
                            )
                        )
                    si.on_wait = keep
                    inst.sync_info = si
                    changed = True
                new_insts.append(inst)
            if changed:
                bb.instructions = new_insts


def _prep_inputs(x, weight, bias, step_scales):
    x = np.asarray(x, dtype=np.float32).reshape(M, K)
    weight = np.asarray(weight, dtype=np.float32)
    bias = np.asarray(bias, dtype=np.float32)
    step_scales = np.asarray(step_scales, dtype=np.float32)

    xt16 = np.ascontiguousarray(x.T.astype(np.float16))            # [K, M]

    s_eff = (step_scales + np.float32(EPS)).astype(np.float32)     # [G, N]
    recip = (np.float32(1.0) / s_eff).astype(np.float32)           # [G, N]
    # wm[k, n] = w.T[k, n] * recip[k//GS, n] + MAGIC, all IEEE fp32 — matches
    # the arithmetic the DVE would do, so rounding is bit-identical.
    w_t = np.ascontiguousarray(weight.T).reshape(G, GS, N)         # [G, GS, N]
    wm_full = (w_t * recip[:, None, :] + np.float32(MAGIC)).astype(np.float32)
    wm_full = wm_full.reshape(K, N)

    s16 = s_eff.astype(np.float16)                                 # [G, N]
    in_maps = []
    for c in range(NCORES):
        sl = slice(c * NS, (c + 1) * NS)
        in_maps.append(
            {
                "xt16": xt16,
                "wm": np.ascontiguousarray(wm_full[:, sl]),
                "srow16": np.ascontiguousarray(s16[:, sl].reshape(1, G * NS)),
                "brep": np.ascontiguousarray(bias[sl].reshape(NT, 128).T),
            }
        )
    return in_maps


def run_on_hw(x, weight, bias, step_scales, trace=False, **kw):
    from concourse.bass_utils import run_bass_kernel_spmd

    if "nc" not in _NC_CACHE:
        _NC_CACHE["nc"] = _build_nc()
    nc = _NC_CACHE["nc"]
    in_maps = _prep_inputs(x, weight, bias, step_scales)
    res = run_bass_kernel_spmd(
        nc, in_maps, core_ids=list(range(NCORES)), trace=trace, **kw
    )
    out_t = np.concatenate([res.results[c]["out_t"] for c in range(NCORES)], axis=0)
    out = np.ascontiguousarray(out_t.T).reshape(B, S, N)
    return out, res


def kernel(x, weight, bias, step_scales):
    out, _ = run_on_hw(x, weight, bias, step_scales, trace=False)
    return out
